# revision 25
# baseline (speedup 1.0000x reference)
"""Trainium2 Bass kernel for exponential smoothing (EMA over time).

Math: out[b,t,h,d] = w_h^{t+1} v0[h,d] + sum_{j<=t} (1-w_h) w_h^{t-j} x[b,j,h,d]
(w = sigmoid(smoothing_weight)), i.e. the scan s_t = w s_{t-1} + (1-w) x_t with
s_{-1} = v0.

Kernel strategy (per core, data-parallel over batch: 16 batches / 8 cores,
2 per core):
  - Time is processed in chunks of C=127. Each chunk step runs 8 per-head
    matmuls whose rhs spans BOTH of the core's batches ([128 x (2,64)],
    N=128): rhs row 0 = carry row, rows 1..127 = x rows; lhsT packs the
    decay column w^{p+1} (for the carry) on top of the lower-triangular
    smoothing weights (1-w) w^{p-j}. The *corrected* last output row of a
    chunk IS the carry for the next chunk: cross-chunk propagation is one
    fused [1,1024] PSUM->SBUF row copy per chunk.
  - lhsT columns are permuted so the chunk's last output row sits at PSUM
    partition 0 (engine APs must start 32-aligned); the out-DMA un-permutes.
  - The 33-step carry chain is broken into 4 independent segments: segments
    1..3 re-derive their incoming carry from 2 warm-up chunks computed with
    a zero carry (EMA influence decays as w^dt; w<=~0.95 -> w^254 ~ 1e-5,
    far below bf16 noise). This gives 4 concurrent chains so the PE never
    waits long on a single carry round-trip.
  - 4096 = 32*127 + 32: 32 full chunks + one 32-row tail chunk.
  - Inputs load contiguously via HWDGE (full 128-partition APs with one
    overlapping predecessor row -- misaligned SBUF DMAs serialize onto one
    SDMA engine), are cast f32->bf16 on ACT, matmuls run in bf16 (fp32 PSUM
    accumulate), output evicts to f32 (one fused ACT op per chunk) and
    stores contiguously.
"""

import numpy as np

B, T, H, D = 16, 4096, 8, 64
HD = H * D                    # 512
C = 127                       # chunk length (1 row reserved for the carry)
NFULL = T // C                # 32 full chunks
REM = T - NFULL * C           # 32-row tail chunk
GROUPS = NFULL // 4           # 8 groups of 4 chunks
NCORES = 8
BPC = B // NCORES             # batches per core
SEG_STARTS = [9, 17, 25]      # segment-start chunks (segments 1..3)

COMPUTE_DTYPE = "bf16"        # "bf16" | "fp32"

_cache = {}


def _host_constants(smoothing_weight, v0, np_cdtype):
    """Parameter-derived constants, computed in fp64 on host."""
    w = 1.0 / (1.0 + np.exp(-smoothing_weight.astype(np.float64)))  # [H,1]
    w = w[:, 0]

    def make_lhsT(n):
        # [H, n+1, n]; row 0 = w^(p+1) (carry decay), row 1+j = (1-w) w^(p-j)
        lt = np.zeros((H, n + 1, n), dtype=np.float64)
        p = np.arange(n)
        for hh in range(H):
            lt[hh, 0, :] = w[hh] ** (p + 1)
            for j in range(n):
                lt[hh, 1 + j, j:] = (1.0 - w[hh]) * w[hh] ** (p[j:] - j)
        return lt.astype(np_cdtype)

    wt = make_lhsT(C)          # [H, 128, 127]
    # permute out rows: [last, 0..last-1] so the carry row lands at PSUM
    # partition 0 (aligned); the out-DMA un-permutes
    wt = np.concatenate([wt[:, :, C - 1:], wt[:, :, :C - 1]], axis=2)
    wt2 = make_lhsT(REM)       # [H, 33, 32] (tail: no carry out, unpermuted)
    # pad M to 128 (zero column): Fast Weight Load needs NumWeights == 128;
    # the extra PSUM row is never read
    wt = np.concatenate([wt, np.zeros((H, C + 1, 1), wt.dtype)], axis=2)
    # [K, H, M] layout so the on-chip weight DMA is contiguous per partition
    wt = np.ascontiguousarray(wt.transpose(1, 0, 2))    # [128, 8, 128]
    wt2 = np.ascontiguousarray(wt2.transpose(1, 0, 2))  # [33, 8, 32]
    wt0 = wt.copy()
    wt0[0, :, :] = 0            # K-row 0 (carry) zeroed: warm-up chunk A
    v0row = v0.reshape(1, HD).astype(np_cdtype)   # [1, 512]
    return wt, wt0, wt2, v0row


def _build_program(cdtype_name):
    import concourse.bass as bass
    import concourse.tile as tile
    from concourse import bacc, mybir
    from contextlib import ExitStack

    cdtype = mybir.dt.bfloat16 if cdtype_name == "bf16" else mybir.dt.float32
    f32 = mybir.dt.float32

    nc = bacc.Bacc("TRN2", target_bir_lowering=False, debug=False,
                   num_devices=NCORES)

    x_d = nc.dram_tensor("x", [BPC, T, HD], f32, kind="ExternalInput").ap()
    wt_d = nc.dram_tensor("wt", [C + 1, H, C + 1], cdtype,
                          kind="ExternalInput").ap()
    wt0_d = nc.dram_tensor("wt0", [C + 1, H, C + 1], cdtype,
                           kind="ExternalInput").ap()
    wt2_d = nc.dram_tensor("wt2", [REM + 1, H, REM], cdtype,
                           kind="ExternalInput").ap()
    v0_d = nc.dram_tensor("v0r", [1, HD], cdtype, kind="ExternalInput").ap()
    out_d = nc.dram_tensor("out", [BPC, T, HD], f32, kind="ExternalOutput").ap()

    with tile.TileContext(nc) as tc, ExitStack() as ctx:
        consts = ctx.enter_context(tc.tile_pool(name="consts", bufs=1))
        in_pool = ctx.enter_context(tc.tile_pool(name="inp", bufs=5))
        in2_pool = ctx.enter_context(tc.tile_pool(name="inp2", bufs=1))
        warm_pool = ctx.enter_context(tc.tile_pool(name="warm", bufs=3))
        out_pool = ctx.enter_context(tc.tile_pool(name="outp", bufs=3))
        out2_pool = ctx.enter_context(tc.tile_pool(name="outp2", bufs=2))
        stage_pool = ctx.enter_context(tc.tile_pool(name="stg", bufs=3))
        psum_pool = ctx.enter_context(tc.tile_pool(name="psum", bufs=3,
                                                   space="PSUM"))
        psum2_pool = ctx.enter_context(tc.tile_pool(name="psum2", bufs=1,
                                                    space="PSUM"))

        # --- constants (DMAs issued inside load_group0 for trigger order) ---
        wt_s = consts.tile([C + 1, H, C + 1], cdtype)   # [128, 8, 128]
        wt0_s = consts.tile([C + 1, H, C + 1], cdtype)  # warm-up A weights
        wt2_s = consts.tile([REM + 1, H, REM], cdtype)  # [33, 8, 32]
        v0_s = consts.tile([1, HD], cdtype)

        # --- tile handles (batch-fused: free axis = (chunk, b, hd)) ---
        in_tiles = {g: in_pool.tile([C + 1, 4, BPC, HD], cdtype, tag="in",
                                    name=f"in_{g}")
                    for g in range(GROUPS)}
        in2_tile = in2_pool.tile([REM + 1, BPC, HD], cdtype, tag="in2",
                                 name="in2")

        def bhd(ap):
            # view a [p, b, (h d)] slice as [p, h, b, d] (PSUM layout order)
            return ap.rearrange("p b (h d) -> p h b d", h=H)

        def load_group(g):
            # Emitted BEFORE any carry copy that targets this tile's row 0
            # (the cast covers the full tile; Tile orders same-region writes
            # by program order). Full-128-partition DMAs with an overlapping
            # predecessor row keep the SBUF side port-group aligned.
            it = in_tiles[g]
            stg = stage_pool.tile([C + 1, 4, BPC, HD], f32, tag="stg")
            for b in range(BPC):
                xb = x_d[b]
                src = bass.AP(
                    tensor=xb.tensor,
                    offset=xb.offset + (4 * C * g - 1) * HD,
                    ap=[[HD, C + 1], [C * HD, 4], [1, HD]],
                )
                nc.sync.dma_start(out=stg[:, :, b, :], in_=src)
            nc.gpsimd.tensor_copy(it[:, :, :, :], stg[:, :, :, :])

        def load_group0():
            # Fast start: per-chunk loads + casts so chunk 0's matmuls can
            # begin as soon as ~256KB has landed (k0 DMAs + v0 + weights are
            # the very first triggers on the serial SP queue). Chunk 0 has
            # no predecessor row: rows split into an unaligned 31-row piece,
            # an aligned 96-row piece, and a junk row 0.
            it = in_tiles[0]
            stg = stage_pool.tile([C + 1, 4, BPC, HD], f32, tag="stg")

            def src_b2(row0, nrows):
                return bass.AP(
                    tensor=x_d.tensor,
                    offset=x_d.offset + row0 * HD,
                    ap=[[HD, nrows], [T * HD, BPC], [1, HD]],
                )
            nc.sync.dma_start(out=stg[0:1, 0, :, :], in_=src_b2(0, 1))
            nc.sync.dma_start(out=stg[1:32, 0, :, :], in_=src_b2(0, 31))
            nc.sync.dma_start(out=stg[32:C + 1, 0, :, :], in_=src_b2(31, 96))
            nc.sync.dma_start(out=v0_s[:], in_=v0_d[:])
            nc.sync.dma_start(out=wt_s[:], in_=wt_d)
            nc.scalar.copy(it[:, 0, :, :], stg[:, 0, :, :])
            for b in range(BPC):
                nc.vector.tensor_copy(it[0:1, 0, b, :], v0_s[:])
            for k in range(1, 4):
                nc.sync.dma_start(out=stg[:, k, :, :],
                                  in_=src_b2(C * k - 1, C + 1))
                nc.scalar.copy(it[:, k, :, :], stg[:, k, :, :])
            nc.sync.dma_start(out=wt2_s[:], in_=wt2_d)
            nc.sync.dma_start(out=wt0_s[:], in_=wt0_d)

        def chunk_step(rhs_view, carry_dst, evict_to=None, weights=None,
                       carry_eng=None):
            # one chunk: 8 batch-fused matmuls -> fused carry copy ->
            # (optional) fused eviction. rhs_view: [128, BPC, HD] bf16.
            w = wt_s if weights is None else weights
            ps = psum_pool.tile([C + 1, H, BPC, D], f32, tag="ps")
            for hh in range(H):
                nc.tensor.matmul(
                    out=ps[:, hh, :, :],
                    lhsT=w[:, hh, :],
                    rhs=rhs_view[:, :, hh * D:(hh + 1) * D],
                    start=True, stop=True,
                )
            if carry_dst is not None:
                ce = nc.vector.tensor_copy if carry_eng is None else carry_eng
                ce(bhd(carry_dst), ps[0:1, :, :, :])
            if evict_to is not None:
                nc.scalar.copy(bhd(evict_to), ps[0:C, :, :, :])

        def warmup(seg):
            # two zero-carry warm-up chunks re-deriving the carry into
            # SEG_STARTS[seg]; outputs are discarded.
            c0 = SEG_STARTS[seg]
            wA = c0 - 2
            wtile = warm_pool.tile([C + 1, 2, BPC, HD], cdtype, tag="warm",
                                   name=f"warm_{seg}")
            stg = stage_pool.tile([C + 1, 2, BPC, HD], f32, tag="stg")
            for kk in range(2):
                src = bass.AP(
                    tensor=x_d.tensor,
                    offset=x_d.offset + ((wA + kk) * C - 1) * HD,
                    ap=[[HD, C + 1], [T * HD, BPC], [1, HD]],
                )
                nc.sync.dma_start(out=stg[:, kk, :, :], in_=src)
            nc.gpsimd.tensor_copy(wtile[:, :, :, :], stg[:, :, :, :])
            # chunk A uses weights with a zeroed carry K-row, so its junk
            # row 0 contributes nothing (no memset, no chain-engine use)
            chunk_step(wtile[:, 0, :, :], wtile[0:1, 1, :, :],
                       weights=wt0_s, carry_eng=nc.scalar.copy)
            g_s, k_s = divmod(c0, 4)
            chunk_step(wtile[:, 1, :, :], in_tiles[g_s][0:1, k_s, :, :],
                       carry_eng=nc.scalar.copy)

        # prologue: first two groups + the segment-start groups (warm-up
        # chains must begin early)
        load_group0()
        load_group(1)
        seg_of_group = {}
        for s, c0 in enumerate(SEG_STARTS):
            seg_of_group[c0 // 4] = s

        loaded = {0, 1}

        def ensure_loaded(g):
            if g in loaded:
                return
            loaded.add(g)
            load_group(g)

        seg_ends = set(c - 1 for c in SEG_STARTS)
        for g in range(GROUPS):
            ensure_loaded(min(g + 2, GROUPS - 1))
            ensure_loaded(min(g + 3, GROUPS - 1))
            it = in_tiles[g]
            ot = out_pool.tile([C, 4, BPC, HD], f32, tag="out",
                               name=f"ot_{g}")
            for k in range(4):
                chunk = 4 * g + k
                if chunk in seg_ends:
                    carry_dst = None        # next segment re-derives it
                elif k < 3:
                    carry_dst = in_tiles[g][0:1, k + 1, :, :]
                elif g < GROUPS - 1:
                    carry_dst = in_tiles[g + 1][0:1, 0, :, :]
                else:
                    carry_dst = in2_tile[0:1, :, :]
                chunk_step(it[:, k, :, :], carry_dst, ot[:, k, :, :])
                # inject warm-up chains once their data can be in flight,
                # after already-ready work in the engine FIFOs
                if chunk == 1:
                    warmup(0)
                elif chunk == 5:
                    warmup(1)
                elif chunk == 13:
                    warmup(2)
            for b in range(BPC):
                dstv = out_d[b, 4 * C * g: 4 * C * (g + 1), :] \
                    .rearrange("(k p) c -> p k c", p=C)
                # un-permute: ot partition 0 = chunk's last time row
                nc.sync.dma_start(out=dstv[0:C - 1, :, :],
                                  in_=ot[1:C, :, b, :])
                nc.sync.dma_start(out=dstv[C - 1:C, :, :],
                                  in_=ot[0:1, :, b, :])

            if g == 4:
                # tail x rows; full-tile cast (row 0 = junk predecessor row)
                # emitted BEFORE g=7's carry copy targets in2_tile row 0
                stg2 = stage_pool.tile([REM + 1, BPC, HD], f32, tag="stg")
                src = bass.AP(
                    tensor=x_d.tensor,
                    offset=x_d.offset + (NFULL * C - 1) * HD,
                    ap=[[HD, REM + 1], [T * HD, BPC], [1, HD]],
                )
                nc.sync.dma_start(out=stg2[:, :, :], in_=src)
                nc.gpsimd.tensor_copy(in2_tile[:, :, :], stg2[:, :, :])

        # --- tail chunk (32 rows) ---
        ps2 = psum2_pool.tile([REM, H, BPC, D], f32, tag="ps2")
        for hh in range(H):
            nc.tensor.matmul(
                out=ps2[:, hh, :, :],
                lhsT=wt2_s[:, hh, :],
                rhs=in2_tile[:, :, hh * D:(hh + 1) * D],
                start=True, stop=True,
            )
        ot2 = out2_pool.tile([REM, BPC, HD], f32, tag="out2", name="ot2")
        nc.scalar.copy(bhd(ot2[:, :, :]), ps2[0:REM, :, :, :])
        for b in range(BPC):
            nc.sync.dma_start(out=out_d[b, NFULL * C:, :], in_=ot2[:, b, :])

    nc.compile()
    return nc


def _get_program():
    key = COMPUTE_DTYPE
    if key not in _cache:
        _cache[key] = _build_program(key)
    return _cache[key]


def kernel(values, smoothing_weight, v0):
    import ml_dtypes
    from concourse.bass_utils import run_bass_kernel_spmd

    np_cdtype = ml_dtypes.bfloat16 if COMPUTE_DTYPE == "bf16" else np.float32
    wt, wt0, wt2, v0row = _host_constants(smoothing_weight, v0, np_cdtype)

    nc = _get_program()
    x = np.ascontiguousarray(values.astype(np.float32).reshape(B, T, HD))
    in_maps = []
    for core in range(NCORES):
        shard = np.ascontiguousarray(x[core * BPC:(core + 1) * BPC])
        in_maps.append({"x": shard, "wt": wt, "wt0": wt0, "wt2": wt2,
                        "v0r": v0row})

    res = run_bass_kernel_spmd(nc, in_maps, list(range(NCORES)))
    outs = [res.results[i]["out"].reshape(BPC, T, H, D)
            for i in range(NCORES)]
    return np.concatenate(outs, axis=0).astype(np.float32)


# revision 26
# speedup vs baseline: 1.3445x; 1.3445x over previous
"""Trainium2 Bass kernel for exponential smoothing (EMA over time).

Math: out[b,t,h,d] = w_h^{t+1} v0[h,d] + sum_{j<=t} (1-w_h) w_h^{t-j} x[b,j,h,d]
(w = sigmoid(smoothing_weight)), i.e. the scan s_t = w s_{t-1} + (1-w) x_t with
s_{-1} = v0.

Kernel strategy (per core, data-parallel over batch: 16 batches / 8 cores,
2 per core):
  - Time is processed in chunks of C=127. Each chunk step runs 8 per-head
    matmuls whose rhs spans BOTH of the core's batches ([128 x (2,64)],
    N=128): rhs row 0 = carry row, rows 1..127 = x rows; lhsT packs the
    decay column w^{p+1} (for the carry) on top of the lower-triangular
    smoothing weights (1-w) w^{p-j}. The *corrected* last output row of a
    chunk IS the carry for the next chunk: cross-chunk propagation is one
    fused [1,1024] PSUM->SBUF row copy per chunk.
  - lhsT columns are permuted so the chunk's last output row sits at PSUM
    partition 0 (engine APs must start 32-aligned); the out-DMA un-permutes.
  - The 33-step carry chain is broken into 4 independent segments: segments
    1..3 re-derive their incoming carry from 2 warm-up chunks computed with
    a zero carry (EMA influence decays as w^dt; w<=~0.95 -> w^254 ~ 1e-5,
    far below bf16 noise). This gives 4 concurrent chains so the PE never
    waits long on a single carry round-trip.
  - 4096 = 32*127 + 32: 32 full chunks + one 32-row tail chunk.
  - Inputs load contiguously via HWDGE (full 128-partition APs with one
    overlapping predecessor row -- misaligned SBUF DMAs serialize onto one
    SDMA engine), are cast f32->bf16 on ACT, matmuls run in bf16 (fp32 PSUM
    accumulate), output evicts to f32 (one fused ACT op per chunk) and
    stores contiguously.
"""

import numpy as np

B, T, H, D = 16, 4096, 8, 64
HD = H * D                    # 512
C = 127                       # chunk length (1 row reserved for the carry)
NFULL = T // C                # 32 full chunks
REM = T - NFULL * C           # 32-row tail chunk
GROUPS = NFULL // 4           # 8 groups of 4 chunks
NCORES = 8
BPC = B // NCORES             # batches per core
SEG_STARTS = [9, 17, 25]      # segment-start chunks (segments 1..3)

COMPUTE_DTYPE = "bf16"        # "bf16" | "fp32"

_cache = {}


def _host_constants(smoothing_weight, v0, np_cdtype):
    """Parameter-derived constants, computed in fp64 on host."""
    w = 1.0 / (1.0 + np.exp(-smoothing_weight.astype(np.float64)))  # [H,1]
    w = w[:, 0]

    def make_lhsT(n):
        # [H, n+1, n]; row 0 = w^(p+1) (carry decay), row 1+j = (1-w) w^(p-j)
        lt = np.zeros((H, n + 1, n), dtype=np.float64)
        p = np.arange(n)
        for hh in range(H):
            lt[hh, 0, :] = w[hh] ** (p + 1)
            for j in range(n):
                lt[hh, 1 + j, j:] = (1.0 - w[hh]) * w[hh] ** (p[j:] - j)
        return lt.astype(np_cdtype)

    wt = make_lhsT(C)          # [H, 128, 127]
    # permute out rows: [last, 0..last-1] so the carry row lands at PSUM
    # partition 0 (aligned); the out-DMA un-permutes
    wt = np.concatenate([wt[:, :, C - 1:], wt[:, :, :C - 1]], axis=2)
    wt2 = make_lhsT(REM)       # [H, 33, 32] (tail: no carry out, unpermuted)
    # pad M to 128 (zero column): Fast Weight Load needs NumWeights == 128;
    # the extra PSUM row is never read
    wt = np.concatenate([wt, np.zeros((H, C + 1, 1), wt.dtype)], axis=2)
    # [K, H, M] layout so the on-chip weight DMA is contiguous per partition
    wt = np.ascontiguousarray(wt.transpose(1, 0, 2))    # [128, 8, 128]
    wt2 = np.ascontiguousarray(wt2.transpose(1, 0, 2))  # [33, 8, 32]
    wt0 = wt.copy()
    wt0[0, :, :] = 0            # K-row 0 (carry) zeroed: warm-up chunk A
    v0row = v0.reshape(1, HD).astype(np_cdtype)   # [1, 512]
    return wt, wt0, wt2, v0row


def _build_program(cdtype_name):
    import concourse.bass as bass
    import concourse.tile as tile
    from concourse import bacc, mybir
    from contextlib import ExitStack

    cdtype = mybir.dt.bfloat16 if cdtype_name == "bf16" else mybir.dt.float32
    f32 = mybir.dt.float32

    nc = bacc.Bacc("TRN2", target_bir_lowering=False, debug=False,
                   num_devices=NCORES)

    x_d = nc.dram_tensor("x", [BPC, T, HD], f32, kind="ExternalInput").ap()
    wt_d = nc.dram_tensor("wt", [C + 1, H, C + 1], cdtype,
                          kind="ExternalInput").ap()
    wt0_d = nc.dram_tensor("wt0", [C + 1, H, C + 1], cdtype,
                           kind="ExternalInput").ap()
    wt2_d = nc.dram_tensor("wt2", [REM + 1, H, REM], cdtype,
                           kind="ExternalInput").ap()
    v0_d = nc.dram_tensor("v0r", [1, HD], cdtype, kind="ExternalInput").ap()
    out_d = nc.dram_tensor("out", [BPC, T, HD], f32, kind="ExternalOutput").ap()

    with tile.TileContext(nc) as tc, ExitStack() as ctx:
        consts = ctx.enter_context(tc.tile_pool(name="consts", bufs=1))
        in_pool = ctx.enter_context(tc.tile_pool(name="inp", bufs=5))
        in2_pool = ctx.enter_context(tc.tile_pool(name="inp2", bufs=1))
        warm_pool = ctx.enter_context(tc.tile_pool(name="warm", bufs=3))
        out_pool = ctx.enter_context(tc.tile_pool(name="outp", bufs=3))
        out2_pool = ctx.enter_context(tc.tile_pool(name="outp2", bufs=2))
        stage_pool = ctx.enter_context(tc.tile_pool(name="stg", bufs=3))
        psum_pool = ctx.enter_context(tc.tile_pool(name="psum", bufs=3,
                                                   space="PSUM"))
        psum2_pool = ctx.enter_context(tc.tile_pool(name="psum2", bufs=1,
                                                    space="PSUM"))

        # --- constants (DMAs issued inside load_group0 for trigger order) ---
        wt_s = consts.tile([C + 1, H, C + 1], cdtype)   # [128, 8, 128]
        wt0_s = consts.tile([C + 1, H, C + 1], cdtype)  # warm-up A weights
        wt2_s = consts.tile([REM + 1, H, REM], cdtype)  # [33, 8, 32]
        v0_s = consts.tile([1, HD], cdtype)

        # --- tile handles (batch-fused: free axis = (chunk, b, hd)) ---
        in_tiles = {g: in_pool.tile([C + 1, 4, BPC, HD], cdtype, tag="in",
                                    name=f"in_{g}")
                    for g in range(GROUPS)}
        in2_tile = in2_pool.tile([REM + 1, BPC, HD], cdtype, tag="in2",
                                 name="in2")

        def bhd(ap):
            # view a [p, b, (h d)] slice as [p, h, b, d] (PSUM layout order)
            return ap.rearrange("p b (h d) -> p h b d", h=H)

        def load_group(g):
            # Emitted BEFORE any carry copy that targets this tile's row 0
            # (the cast covers the full tile; Tile orders same-region writes
            # by program order). Full-128-partition DMAs with an overlapping
            # predecessor row keep the SBUF side port-group aligned.
            it = in_tiles[g]
            stg = stage_pool.tile([C + 1, 4, BPC, HD], f32, tag="stg")
            for b in range(BPC):
                xb = x_d[b]
                src = bass.AP(
                    tensor=xb.tensor,
                    offset=xb.offset + (4 * C * g - 1) * HD,
                    ap=[[HD, C + 1], [C * HD, 4], [1, HD]],
                )
                nc.sync.dma_start(out=stg[:, :, b, :], in_=src)
            nc.scalar.copy(it[:, :, :, :], stg[:, :, :, :])

        def load_group0():
            # Fast start: per-chunk loads + casts so chunk 0's matmuls can
            # begin as soon as ~256KB has landed (k0 DMAs + v0 + weights are
            # the very first triggers on the serial SP queue). Chunk 0 has
            # no predecessor row: rows split into an unaligned 31-row piece,
            # an aligned 96-row piece, and a junk row 0.
            it = in_tiles[0]
            stg = stage_pool.tile([C + 1, 4, BPC, HD], f32, tag="stg")

            def src_b2(row0, nrows):
                return bass.AP(
                    tensor=x_d.tensor,
                    offset=x_d.offset + row0 * HD,
                    ap=[[HD, nrows], [T * HD, BPC], [1, HD]],
                )
            nc.sync.dma_start(out=stg[0:1, 0, :, :], in_=src_b2(0, 1))
            nc.sync.dma_start(out=stg[1:32, 0, :, :], in_=src_b2(0, 31))
            nc.sync.dma_start(out=stg[32:C + 1, 0, :, :], in_=src_b2(31, 96))
            nc.sync.dma_start(out=v0_s[:], in_=v0_d[:])
            nc.sync.dma_start(out=wt_s[:], in_=wt_d)
            nc.scalar.copy(it[:, 0, :, :], stg[:, 0, :, :])
            for b in range(BPC):
                nc.vector.tensor_copy(it[0:1, 0, b, :], v0_s[:])
            for k in range(1, 4):
                nc.sync.dma_start(out=stg[:, k, :, :],
                                  in_=src_b2(C * k - 1, C + 1))
                nc.scalar.copy(it[:, k, :, :], stg[:, k, :, :])
            nc.sync.dma_start(out=wt2_s[:], in_=wt2_d)
            nc.sync.dma_start(out=wt0_s[:], in_=wt0_d)

        def chunk_step(rhs_view, carry_dst, evict_to=None, weights=None,
                       carry_eng=None):
            # one chunk: 8 batch-fused matmuls -> fused carry copy ->
            # (optional) fused eviction. rhs_view: [128, BPC, HD] bf16.
            w = wt_s if weights is None else weights
            ps = psum_pool.tile([C + 1, H, BPC, D], f32, tag="ps")
            for hh in range(H):
                nc.tensor.matmul(
                    out=ps[:, hh, :, :],
                    lhsT=w[:, hh, :],
                    rhs=rhs_view[:, :, hh * D:(hh + 1) * D],
                    start=True, stop=True,
                )
            if carry_dst is not None:
                ce = nc.vector.tensor_copy if carry_eng is None else carry_eng
                ce(bhd(carry_dst), ps[0:1, :, :, :])
            if evict_to is not None:
                nc.scalar.copy(bhd(evict_to), ps[0:C, :, :, :])

        def warmup(seg):
            # two zero-carry warm-up chunks re-deriving the carry into
            # SEG_STARTS[seg]; outputs are discarded.
            c0 = SEG_STARTS[seg]
            wA = c0 - 2
            wtile = warm_pool.tile([C + 1, 2, BPC, HD], cdtype, tag="warm",
                                   name=f"warm_{seg}")
            stg = stage_pool.tile([C + 1, 2, BPC, HD], f32, tag="stg")
            for kk in range(2):
                src = bass.AP(
                    tensor=x_d.tensor,
                    offset=x_d.offset + ((wA + kk) * C - 1) * HD,
                    ap=[[HD, C + 1], [T * HD, BPC], [1, HD]],
                )
                nc.sync.dma_start(out=stg[:, kk, :, :], in_=src)
            nc.scalar.copy(wtile[:, :, :, :], stg[:, :, :, :])
            # chunk A uses weights with a zeroed carry K-row, so its junk
            # row 0 contributes nothing (no memset, no chain-engine use)
            chunk_step(wtile[:, 0, :, :], wtile[0:1, 1, :, :],
                       weights=wt0_s, carry_eng=nc.scalar.copy)
            g_s, k_s = divmod(c0, 4)
            chunk_step(wtile[:, 1, :, :], in_tiles[g_s][0:1, k_s, :, :],
                       carry_eng=nc.scalar.copy)

        # prologue: first two groups + the segment-start groups (warm-up
        # chains must begin early)
        load_group0()
        load_group(1)
        seg_of_group = {}
        for s, c0 in enumerate(SEG_STARTS):
            seg_of_group[c0 // 4] = s

        loaded = {0, 1}

        def ensure_loaded(g):
            if g in loaded:
                return
            loaded.add(g)
            load_group(g)

        seg_ends = set(c - 1 for c in SEG_STARTS)
        for g in range(GROUPS):
            ensure_loaded(min(g + 2, GROUPS - 1))
            ensure_loaded(min(g + 3, GROUPS - 1))
            it = in_tiles[g]
            ot = out_pool.tile([C, 4, BPC, HD], f32, tag="out",
                               name=f"ot_{g}")
            for k in range(4):
                chunk = 4 * g + k
                if chunk in seg_ends:
                    carry_dst = None        # next segment re-derives it
                elif k < 3:
                    carry_dst = in_tiles[g][0:1, k + 1, :, :]
                elif g < GROUPS - 1:
                    carry_dst = in_tiles[g + 1][0:1, 0, :, :]
                else:
                    carry_dst = in2_tile[0:1, :, :]
                chunk_step(it[:, k, :, :], carry_dst, ot[:, k, :, :])
                # inject warm-up chains once their data can be in flight,
                # after already-ready work in the engine FIFOs
                if chunk == 1:
                    warmup(0)
                elif chunk == 5:
                    warmup(1)
                elif chunk == 13:
                    warmup(2)
            for b in range(BPC):
                dstv = out_d[b, 4 * C * g: 4 * C * (g + 1), :] \
                    .rearrange("(k p) c -> p k c", p=C)
                # un-permute: ot partition 0 = chunk's last time row
                nc.sync.dma_start(out=dstv[0:C - 1, :, :],
                                  in_=ot[1:C, :, b, :])
                nc.sync.dma_start(out=dstv[C - 1:C, :, :],
                                  in_=ot[0:1, :, b, :])

            if g == 4:
                # tail x rows; full-tile cast (row 0 = junk predecessor row)
                # emitted BEFORE g=7's carry copy targets in2_tile row 0
                stg2 = stage_pool.tile([REM + 1, BPC, HD], f32, tag="stg")
                src = bass.AP(
                    tensor=x_d.tensor,
                    offset=x_d.offset + (NFULL * C - 1) * HD,
                    ap=[[HD, REM + 1], [T * HD, BPC], [1, HD]],
                )
                nc.sync.dma_start(out=stg2[:, :, :], in_=src)
                nc.scalar.copy(in2_tile[:, :, :], stg2[:, :, :])

        # --- tail chunk (32 rows) ---
        ps2 = psum2_pool.tile([REM, H, BPC, D], f32, tag="ps2")
        for hh in range(H):
            nc.tensor.matmul(
                out=ps2[:, hh, :, :],
                lhsT=wt2_s[:, hh, :],
                rhs=in2_tile[:, :, hh * D:(hh + 1) * D],
                start=True, stop=True,
            )
        ot2 = out2_pool.tile([REM, BPC, HD], f32, tag="out2", name="ot2")
        nc.scalar.copy(bhd(ot2[:, :, :]), ps2[0:REM, :, :, :])
        for b in range(BPC):
            nc.sync.dma_start(out=out_d[b, NFULL * C:, :], in_=ot2[:, b, :])

    nc.compile()
    return nc


def _get_program():
    key = COMPUTE_DTYPE
    if key not in _cache:
        _cache[key] = _build_program(key)
    return _cache[key]


def kernel(values, smoothing_weight, v0):
    import ml_dtypes
    from concourse.bass_utils import run_bass_kernel_spmd

    np_cdtype = ml_dtypes.bfloat16 if COMPUTE_DTYPE == "bf16" else np.float32
    wt, wt0, wt2, v0row = _host_constants(smoothing_weight, v0, np_cdtype)

    nc = _get_program()
    x = np.ascontiguousarray(values.astype(np.float32).reshape(B, T, HD))
    in_maps = []
    for core in range(NCORES):
        shard = np.ascontiguousarray(x[core * BPC:(core + 1) * BPC])
        in_maps.append({"x": shard, "wt": wt, "wt0": wt0, "wt2": wt2,
                        "v0r": v0row})

    res = run_bass_kernel_spmd(nc, in_maps, list(range(NCORES)))
    outs = [res.results[i]["out"].reshape(BPC, T, H, D)
            for i in range(NCORES)]
    return np.concatenate(outs, axis=0).astype(np.float32)


# revision 27
# speedup vs baseline: 1.3835x; 1.0290x over previous
"""Trainium2 Bass kernel for exponential smoothing (EMA over time).

Math: out[b,t,h,d] = w_h^{t+1} v0[h,d] + sum_{j<=t} (1-w_h) w_h^{t-j} x[b,j,h,d]
(w = sigmoid(smoothing_weight)), i.e. the scan s_t = w s_{t-1} + (1-w) x_t with
s_{-1} = v0.

Kernel strategy (per core, data-parallel over batch: 16 batches / 8 cores,
2 per core):
  - Time is processed in chunks of C=127. Each chunk step runs 8 per-head
    matmuls whose rhs spans BOTH of the core's batches ([128 x (2,64)],
    N=128): rhs row 0 = carry row, rows 1..127 = x rows; lhsT packs the
    decay column w^{p+1} (for the carry) on top of the lower-triangular
    smoothing weights (1-w) w^{p-j}. The *corrected* last output row of a
    chunk IS the carry for the next chunk: cross-chunk propagation is one
    fused [1,1024] PSUM->SBUF row copy per chunk.
  - lhsT columns are permuted so the chunk's last output row sits at PSUM
    partition 0 (engine APs must start 32-aligned); the out-DMA un-permutes.
  - The 33-step carry chain is broken into 4 independent segments: segments
    1..3 re-derive their incoming carry from 2 warm-up chunks computed with
    a zero carry (EMA influence decays as w^dt; w<=~0.95 -> w^254 ~ 1e-5,
    far below bf16 noise). This gives 4 concurrent chains so the PE never
    waits long on a single carry round-trip.
  - 4096 = 32*127 + 32: 32 full chunks + one 32-row tail chunk.
  - Inputs load contiguously via HWDGE (full 128-partition APs with one
    overlapping predecessor row -- misaligned SBUF DMAs serialize onto one
    SDMA engine), are cast f32->bf16 on ACT, matmuls run in bf16 (fp32 PSUM
    accumulate), output evicts to f32 (one fused ACT op per chunk) and
    stores contiguously.
"""

import numpy as np

B, T, H, D = 16, 4096, 8, 64
HD = H * D                    # 512
C = 127                       # chunk length (1 row reserved for the carry)
NFULL = T // C                # 32 full chunks
REM = T - NFULL * C           # 32-row tail chunk
GROUPS = NFULL // 4           # 8 groups of 4 chunks
NCORES = 8
BPC = B // NCORES             # batches per core
SEG_STARTS = [9, 17, 25]      # segment-start chunks (segments 1..3)

COMPUTE_DTYPE = "bf16"        # "bf16" | "fp32"

_cache = {}


def _host_constants(smoothing_weight, v0, np_cdtype):
    """Parameter-derived constants, computed in fp64 on host."""
    w = 1.0 / (1.0 + np.exp(-smoothing_weight.astype(np.float64)))  # [H,1]
    w = w[:, 0]

    def make_lhsT(n):
        # [H, n+1, n]; row 0 = w^(p+1) (carry decay), row 1+j = (1-w) w^(p-j)
        lt = np.zeros((H, n + 1, n), dtype=np.float64)
        p = np.arange(n)
        for hh in range(H):
            lt[hh, 0, :] = w[hh] ** (p + 1)
            for j in range(n):
                lt[hh, 1 + j, j:] = (1.0 - w[hh]) * w[hh] ** (p[j:] - j)
        return lt.astype(np_cdtype)

    wt = make_lhsT(C)          # [H, 128, 127]
    # permute out rows: [last, 0..last-1] so the carry row lands at PSUM
    # partition 0 (aligned); the out-DMA un-permutes
    wt = np.concatenate([wt[:, :, C - 1:], wt[:, :, :C - 1]], axis=2)
    wt2 = make_lhsT(REM)       # [H, 33, 32] (tail: no carry out, unpermuted)
    # pad M to 128 (zero column): Fast Weight Load needs NumWeights == 128;
    # the extra PSUM row is never read
    wt = np.concatenate([wt, np.zeros((H, C + 1, 1), wt.dtype)], axis=2)
    # [K, H, M] layout so the on-chip weight DMA is contiguous per partition
    wt = np.ascontiguousarray(wt.transpose(1, 0, 2))    # [128, 8, 128]
    wt2 = np.ascontiguousarray(wt2.transpose(1, 0, 2))  # [33, 8, 32]
    wt0 = wt.copy()
    wt0[0, :, :] = 0            # K-row 0 (carry) zeroed: warm-up chunk A
    v0row = v0.reshape(1, HD).astype(np_cdtype)   # [1, 512]
    return wt, wt0, wt2, v0row


def _build_program(cdtype_name):
    import concourse.bass as bass
    import concourse.tile as tile
    from concourse import bacc, mybir
    from contextlib import ExitStack

    cdtype = mybir.dt.bfloat16 if cdtype_name == "bf16" else mybir.dt.float32
    f32 = mybir.dt.float32

    nc = bacc.Bacc("TRN2", target_bir_lowering=False, debug=False,
                   num_devices=NCORES)

    x_d = nc.dram_tensor("x", [BPC, T, HD], f32, kind="ExternalInput").ap()
    wt_d = nc.dram_tensor("wt", [C + 1, H, C + 1], cdtype,
                          kind="ExternalInput").ap()
    wt0_d = nc.dram_tensor("wt0", [C + 1, H, C + 1], cdtype,
                           kind="ExternalInput").ap()
    wt2_d = nc.dram_tensor("wt2", [REM + 1, H, REM], cdtype,
                           kind="ExternalInput").ap()
    v0_d = nc.dram_tensor("v0r", [1, HD], cdtype, kind="ExternalInput").ap()
    out_d = nc.dram_tensor("out", [BPC, T, HD], f32, kind="ExternalOutput").ap()

    with tile.TileContext(nc) as tc, ExitStack() as ctx:
        consts = ctx.enter_context(tc.tile_pool(name="consts", bufs=1))
        in_pool = ctx.enter_context(tc.tile_pool(name="inp", bufs=5))
        in2_pool = ctx.enter_context(tc.tile_pool(name="inp2", bufs=1))
        warm_pool = ctx.enter_context(tc.tile_pool(name="warm", bufs=3))
        out_pool = ctx.enter_context(tc.tile_pool(name="outp", bufs=3))
        out2_pool = ctx.enter_context(tc.tile_pool(name="outp2", bufs=2))
        stage_pool = ctx.enter_context(tc.tile_pool(name="stg", bufs=3))
        psum_pool = ctx.enter_context(tc.tile_pool(name="psum", bufs=3,
                                                   space="PSUM"))
        psum2_pool = ctx.enter_context(tc.tile_pool(name="psum2", bufs=1,
                                                    space="PSUM"))

        # --- constants (DMAs issued inside load_group0 for trigger order) ---
        wt_s = consts.tile([C + 1, H, C + 1], cdtype)   # [128, 8, 128]
        wt0_s = consts.tile([C + 1, H, C + 1], cdtype)  # warm-up A weights
        wt2_s = consts.tile([REM + 1, H, REM], cdtype)  # [33, 8, 32]
        v0_s = consts.tile([1, HD], cdtype)

        # --- tile handles (batch-fused: free axis = (chunk, b, hd)) ---
        in_tiles = {g: in_pool.tile([C + 1, 4, BPC, HD], cdtype, tag="in",
                                    name=f"in_{g}")
                    for g in range(GROUPS)}
        in2_tile = in2_pool.tile([REM + 1, BPC, HD], cdtype, tag="in2",
                                 name="in2")

        def bhd(ap):
            # view a [p, b, (h d)] slice as [p, h, b, d] (PSUM layout order)
            return ap.rearrange("p b (h d) -> p h b d", h=H)

        def load_group(g):
            # Emitted BEFORE any carry copy that targets this tile's row 0
            # (the cast covers the full tile; Tile orders same-region writes
            # by program order). Full-128-partition DMAs with an overlapping
            # predecessor row keep the SBUF side port-group aligned.
            it = in_tiles[g]
            stg = stage_pool.tile([C + 1, 4, BPC, HD], f32, tag="stg")
            for b in range(BPC):
                xb = x_d[b]
                src = bass.AP(
                    tensor=xb.tensor,
                    offset=xb.offset + (4 * C * g - 1) * HD,
                    ap=[[HD, C + 1], [C * HD, 4], [1, HD]],
                )
                nc.sync.dma_start(out=stg[:, :, b, :], in_=src)
            nc.scalar.copy(it[:, :, :, :], stg[:, :, :, :])

        def load_group0():
            # Fast start: per-chunk loads + casts so chunk 0's matmuls can
            # begin as soon as ~256KB has landed (k0 DMAs + v0 + weights are
            # the very first triggers on the serial SP queue). Chunk 0 has
            # no predecessor row: rows split into an unaligned 31-row piece,
            # an aligned 96-row piece, and a junk row 0.
            it = in_tiles[0]
            stg = stage_pool.tile([C + 1, 4, BPC, HD], f32, tag="stg")

            def src_b2(row0, nrows):
                return bass.AP(
                    tensor=x_d.tensor,
                    offset=x_d.offset + row0 * HD,
                    ap=[[HD, nrows], [T * HD, BPC], [1, HD]],
                )
            nc.sync.dma_start(out=stg[0:1, 0, :, :], in_=src_b2(0, 1))
            nc.sync.dma_start(out=stg[1:32, 0, :, :], in_=src_b2(0, 31))
            nc.sync.dma_start(out=stg[32:C + 1, 0, :, :], in_=src_b2(31, 96))
            nc.sync.dma_start(out=v0_s[:], in_=v0_d[:])
            nc.sync.dma_start(out=wt_s[:], in_=wt_d)
            nc.scalar.copy(it[:, 0, :, :], stg[:, 0, :, :])
            for b in range(BPC):
                nc.vector.tensor_copy(it[0:1, 0, b, :], v0_s[:])
            for k in range(1, 4):
                nc.sync.dma_start(out=stg[:, k, :, :],
                                  in_=src_b2(C * k - 1, C + 1))
                nc.scalar.copy(it[:, k, :, :], stg[:, k, :, :])
            nc.sync.dma_start(out=wt2_s[:], in_=wt2_d)
            nc.sync.dma_start(out=wt0_s[:], in_=wt0_d)

        def chunk_step(rhs_view, carry_dst, evict_to=None, weights=None,
                       carry_eng=None):
            # one chunk: 8 batch-fused matmuls -> fused carry copy ->
            # (optional) fused eviction. rhs_view: [128, BPC, HD] bf16.
            w = wt_s if weights is None else weights
            ps = psum_pool.tile([C + 1, H, BPC, D], f32, tag="ps")
            for hh in range(H):
                nc.tensor.matmul(
                    out=ps[:, hh, :, :],
                    lhsT=w[:, hh, :],
                    rhs=rhs_view[:, :, hh * D:(hh + 1) * D],
                    start=True, stop=True,
                )
            if carry_dst is not None:
                ce = nc.vector.tensor_copy if carry_eng is None else carry_eng
                ce(bhd(carry_dst), ps[0:1, :, :, :])
            if evict_to is not None:
                nc.scalar.copy(bhd(evict_to), ps[0:C, :, :, :])

        def warmup(seg):
            # two zero-carry warm-up chunks re-deriving the carry into
            # SEG_STARTS[seg]; outputs are discarded.
            c0 = SEG_STARTS[seg]
            wA = c0 - 2
            wtile = warm_pool.tile([C + 1, 2, BPC, HD], cdtype, tag="warm",
                                   name=f"warm_{seg}")
            stg = stage_pool.tile([C + 1, 2, BPC, HD], f32, tag="stg")
            for kk in range(2):
                src = bass.AP(
                    tensor=x_d.tensor,
                    offset=x_d.offset + ((wA + kk) * C - 1) * HD,
                    ap=[[HD, C + 1], [T * HD, BPC], [1, HD]],
                )
                nc.sync.dma_start(out=stg[:, kk, :, :], in_=src)
            nc.scalar.copy(wtile[:, :, :, :], stg[:, :, :, :])
            # chunk A uses weights with a zeroed carry K-row, so its junk
            # row 0 contributes nothing (no memset, no chain-engine use)
            chunk_step(wtile[:, 0, :, :], wtile[0:1, 1, :, :],
                       weights=wt0_s)
            g_s, k_s = divmod(c0, 4)
            chunk_step(wtile[:, 1, :, :], in_tiles[g_s][0:1, k_s, :, :])

        # prologue: first two groups + the segment-start groups (warm-up
        # chains must begin early)
        load_group0()
        load_group(1)
        seg_of_group = {}
        for s, c0 in enumerate(SEG_STARTS):
            seg_of_group[c0 // 4] = s

        loaded = {0, 1}

        def ensure_loaded(g):
            if g in loaded:
                return
            loaded.add(g)
            load_group(g)

        seg_ends = set(c - 1 for c in SEG_STARTS)
        for g in range(GROUPS):
            ensure_loaded(min(g + 2, GROUPS - 1))
            ensure_loaded(min(g + 3, GROUPS - 1))
            it = in_tiles[g]
            ot = out_pool.tile([C, 4, BPC, HD], f32, tag="out",
                               name=f"ot_{g}")
            for k in range(4):
                chunk = 4 * g + k
                if chunk in seg_ends:
                    carry_dst = None        # next segment re-derives it
                elif k < 3:
                    carry_dst = in_tiles[g][0:1, k + 1, :, :]
                elif g < GROUPS - 1:
                    carry_dst = in_tiles[g + 1][0:1, 0, :, :]
                else:
                    carry_dst = in2_tile[0:1, :, :]
                chunk_step(it[:, k, :, :], carry_dst, ot[:, k, :, :])
                # inject warm-up chains once their data can be in flight,
                # after already-ready work in the engine FIFOs
                if chunk == 1:
                    warmup(0)
                elif chunk == 5:
                    warmup(1)
                elif chunk == 13:
                    warmup(2)
            for b in range(BPC):
                dstv = out_d[b, 4 * C * g: 4 * C * (g + 1), :] \
                    .rearrange("(k p) c -> p k c", p=C)
                # un-permute: ot partition 0 = chunk's last time row
                nc.sync.dma_start(out=dstv[0:C - 1, :, :],
                                  in_=ot[1:C, :, b, :])
                nc.sync.dma_start(out=dstv[C - 1:C, :, :],
                                  in_=ot[0:1, :, b, :])

            if g == 4:
                # tail x rows; full-tile cast (row 0 = junk predecessor row)
                # emitted BEFORE g=7's carry copy targets in2_tile row 0
                stg2 = stage_pool.tile([REM + 1, BPC, HD], f32, tag="stg")
                src = bass.AP(
                    tensor=x_d.tensor,
                    offset=x_d.offset + (NFULL * C - 1) * HD,
                    ap=[[HD, REM + 1], [T * HD, BPC], [1, HD]],
                )
                nc.sync.dma_start(out=stg2[:, :, :], in_=src)
                nc.scalar.copy(in2_tile[:, :, :], stg2[:, :, :])

        # --- tail chunk (32 rows) ---
        ps2 = psum2_pool.tile([REM, H, BPC, D], f32, tag="ps2")
        for hh in range(H):
            nc.tensor.matmul(
                out=ps2[:, hh, :, :],
                lhsT=wt2_s[:, hh, :],
                rhs=in2_tile[:, :, hh * D:(hh + 1) * D],
                start=True, stop=True,
            )
        ot2 = out2_pool.tile([REM, BPC, HD], f32, tag="out2", name="ot2")
        nc.scalar.copy(bhd(ot2[:, :, :]), ps2[0:REM, :, :, :])
        for b in range(BPC):
            nc.sync.dma_start(out=out_d[b, NFULL * C:, :], in_=ot2[:, b, :])

    nc.compile()
    return nc


def _get_program():
    key = COMPUTE_DTYPE
    if key not in _cache:
        _cache[key] = _build_program(key)
    return _cache[key]


def kernel(values, smoothing_weight, v0):
    import ml_dtypes
    from concourse.bass_utils import run_bass_kernel_spmd

    np_cdtype = ml_dtypes.bfloat16 if COMPUTE_DTYPE == "bf16" else np.float32
    wt, wt0, wt2, v0row = _host_constants(smoothing_weight, v0, np_cdtype)

    nc = _get_program()
    x = np.ascontiguousarray(values.astype(np.float32).reshape(B, T, HD))
    in_maps = []
    for core in range(NCORES):
        shard = np.ascontiguousarray(x[core * BPC:(core + 1) * BPC])
        in_maps.append({"x": shard, "wt": wt, "wt0": wt0, "wt2": wt2,
                        "v0r": v0row})

    res = run_bass_kernel_spmd(nc, in_maps, list(range(NCORES)))
    outs = [res.results[i]["out"].reshape(BPC, T, H, D)
            for i in range(NCORES)]
    return np.concatenate(outs, axis=0).astype(np.float32)


# revision 28
# speedup vs baseline: 1.4383x; 1.0396x over previous
"""Trainium2 Bass kernel for exponential smoothing (EMA over time).

Math: out[b,t,h,d] = w_h^{t+1} v0[h,d] + sum_{j<=t} (1-w_h) w_h^{t-j} x[b,j,h,d]
(w = sigmoid(smoothing_weight)), i.e. the scan s_t = w s_{t-1} + (1-w) x_t with
s_{-1} = v0.

Kernel strategy (per core, data-parallel over batch: 16 batches / 8 cores,
2 per core):
  - Time is processed in chunks of C=127. Each chunk step runs 8 per-head
    matmuls whose rhs spans BOTH of the core's batches ([128 x (2,64)],
    N=128): rhs row 0 = carry row, rows 1..127 = x rows; lhsT packs the
    decay column w^{p+1} (for the carry) on top of the lower-triangular
    smoothing weights (1-w) w^{p-j}. The *corrected* last output row of a
    chunk IS the carry for the next chunk: cross-chunk propagation is one
    fused [1,1024] PSUM->SBUF row copy per chunk.
  - lhsT columns are permuted so the chunk's last output row sits at PSUM
    partition 0 (engine APs must start 32-aligned); the out-DMA un-permutes.
  - The 33-step carry chain is broken into 4 independent segments: segments
    1..3 re-derive their incoming carry from 2 warm-up chunks computed with
    a zero carry (EMA influence decays as w^dt; w<=~0.95 -> w^254 ~ 1e-5,
    far below bf16 noise). This gives 4 concurrent chains so the PE never
    waits long on a single carry round-trip.
  - 4096 = 32*127 + 32: 32 full chunks + one 32-row tail chunk.
  - Inputs load contiguously via HWDGE (full 128-partition APs with one
    overlapping predecessor row -- misaligned SBUF DMAs serialize onto one
    SDMA engine), are cast f32->bf16 on ACT, matmuls run in bf16 (fp32 PSUM
    accumulate), output evicts to f32 (one fused ACT op per chunk) and
    stores contiguously.
"""

import numpy as np

B, T, H, D = 16, 4096, 8, 64
HD = H * D                    # 512
C = 127                       # chunk length (1 row reserved for the carry)
NFULL = T // C                # 32 full chunks
REM = T - NFULL * C           # 32-row tail chunk
GROUPS = NFULL // 4           # 8 groups of 4 chunks
NCORES = 8
BPC = B // NCORES             # batches per core
SEG_STARTS = [9, 17, 25]      # segment-start chunks (segments 1..3)

COMPUTE_DTYPE = "bf16"        # "bf16" | "fp32"

_cache = {}


def _host_constants(smoothing_weight, v0, np_cdtype):
    """Parameter-derived constants, computed in fp64 on host."""
    w = 1.0 / (1.0 + np.exp(-smoothing_weight.astype(np.float64)))  # [H,1]
    w = w[:, 0]

    def make_lhsT(n):
        # [H, n+1, n]; row 0 = w^(p+1) (carry decay), row 1+j = (1-w) w^(p-j)
        lt = np.zeros((H, n + 1, n), dtype=np.float64)
        p = np.arange(n)
        for hh in range(H):
            lt[hh, 0, :] = w[hh] ** (p + 1)
            for j in range(n):
                lt[hh, 1 + j, j:] = (1.0 - w[hh]) * w[hh] ** (p[j:] - j)
        return lt.astype(np_cdtype)

    wt = make_lhsT(C)          # [H, 128, 127]
    # permute out rows: [last, 0..last-1] so the carry row lands at PSUM
    # partition 0 (aligned); the out-DMA un-permutes
    wt = np.concatenate([wt[:, :, C - 1:], wt[:, :, :C - 1]], axis=2)
    wt2 = make_lhsT(REM)       # [H, 33, 32] (tail: no carry out, unpermuted)
    # pad M to 128 (zero column): Fast Weight Load needs NumWeights == 128;
    # the extra PSUM row is never read
    wt = np.concatenate([wt, np.zeros((H, C + 1, 1), wt.dtype)], axis=2)
    # [K, H, M] layout so the on-chip weight DMA is contiguous per partition
    wt = np.ascontiguousarray(wt.transpose(1, 0, 2))    # [128, 8, 128]
    wt2 = np.ascontiguousarray(wt2.transpose(1, 0, 2))  # [33, 8, 32]
    wt0 = wt.copy()
    wt0[0, :, :] = 0            # K-row 0 (carry) zeroed: warm-up chunk A
    v0row = v0.reshape(1, HD).astype(np_cdtype)   # [1, 512]
    return wt, wt0, wt2, v0row


def _build_program(cdtype_name):
    import concourse.bass as bass
    import concourse.tile as tile
    from concourse import bacc, mybir
    from contextlib import ExitStack

    cdtype = mybir.dt.bfloat16 if cdtype_name == "bf16" else mybir.dt.float32
    f32 = mybir.dt.float32

    nc = bacc.Bacc("TRN2", target_bir_lowering=False, debug=False,
                   num_devices=NCORES)

    x_d = nc.dram_tensor("x", [BPC, T, HD], f32, kind="ExternalInput").ap()
    wt_d = nc.dram_tensor("wt", [C + 1, H, C + 1], cdtype,
                          kind="ExternalInput").ap()
    wt0_d = nc.dram_tensor("wt0", [C + 1, H, C + 1], cdtype,
                           kind="ExternalInput").ap()
    wt2_d = nc.dram_tensor("wt2", [REM + 1, H, REM], cdtype,
                           kind="ExternalInput").ap()
    v0_d = nc.dram_tensor("v0r", [1, HD], cdtype, kind="ExternalInput").ap()
    out_d = nc.dram_tensor("out", [BPC, T, HD], f32, kind="ExternalOutput").ap()

    with tile.TileContext(nc) as tc, ExitStack() as ctx:
        consts = ctx.enter_context(tc.tile_pool(name="consts", bufs=1))
        in_pool = ctx.enter_context(tc.tile_pool(name="inp", bufs=5))
        in2_pool = ctx.enter_context(tc.tile_pool(name="inp2", bufs=1))
        warm_pool = ctx.enter_context(tc.tile_pool(name="warm", bufs=3))
        out_pool = ctx.enter_context(tc.tile_pool(name="outp", bufs=3))
        out2_pool = ctx.enter_context(tc.tile_pool(name="outp2", bufs=2))
        stage_pool = ctx.enter_context(tc.tile_pool(name="stg", bufs=3))
        psum_pool = ctx.enter_context(tc.tile_pool(name="psum", bufs=3,
                                                   space="PSUM"))
        psum2_pool = ctx.enter_context(tc.tile_pool(name="psum2", bufs=1,
                                                    space="PSUM"))

        # --- constants (DMAs issued inside load_group0 for trigger order) ---
        wt_s = consts.tile([C + 1, H, C + 1], cdtype)   # [128, 8, 128]
        wt0_s = consts.tile([C + 1, H, C + 1], cdtype)  # warm-up A weights
        wt2_s = consts.tile([REM + 1, H, REM], cdtype)  # [33, 8, 32]
        v0_s = consts.tile([1, HD], cdtype)

        # --- tile handles (batch-fused: free axis = (chunk, b, hd)) ---
        in_tiles = {g: in_pool.tile([C + 1, 4, BPC, HD], cdtype, tag="in",
                                    name=f"in_{g}")
                    for g in range(GROUPS)}
        in2_tile = in2_pool.tile([REM + 1, BPC, HD], cdtype, tag="in2",
                                 name="in2")

        def bhd(ap):
            # view a [p, b, (h d)] slice as [p, h, b, d] (PSUM layout order)
            return ap.rearrange("p b (h d) -> p h b d", h=H)

        def load_group(g):
            # Emitted BEFORE any carry copy that targets this tile's row 0
            # (the cast covers the full tile; Tile orders same-region writes
            # by program order). Full-128-partition DMAs with an overlapping
            # predecessor row keep the SBUF side port-group aligned.
            it = in_tiles[g]
            stg = stage_pool.tile([C + 1, 4, BPC, HD], f32, tag="stg")
            for b in range(BPC):
                xb = x_d[b]
                src = bass.AP(
                    tensor=xb.tensor,
                    offset=xb.offset + (4 * C * g - 1) * HD,
                    ap=[[HD, C + 1], [C * HD, 4], [1, HD]],
                )
                nc.sync.dma_start(out=stg[:, :, b, :], in_=src)
            nc.scalar.copy(it[:, :, :, :], stg[:, :, :, :])

        def load_group0():
            # Fast start: per-chunk loads + casts so chunk 0's matmuls can
            # begin as soon as ~256KB has landed (k0 DMAs + v0 + weights are
            # the very first triggers on the serial SP queue). Chunk 0 has
            # no predecessor row: rows split into an unaligned 31-row piece,
            # an aligned 96-row piece, and a junk row 0.
            it = in_tiles[0]
            stg = stage_pool.tile([C + 1, 4, BPC, HD], f32, tag="stg")

            def src_b2(row0, nrows):
                return bass.AP(
                    tensor=x_d.tensor,
                    offset=x_d.offset + row0 * HD,
                    ap=[[HD, nrows], [T * HD, BPC], [1, HD]],
                )
            nc.sync.dma_start(out=stg[0:1, 0, :, :], in_=src_b2(0, 1))
            nc.sync.dma_start(out=stg[1:32, 0, :, :], in_=src_b2(0, 31))
            nc.sync.dma_start(out=stg[32:C + 1, 0, :, :], in_=src_b2(31, 96))
            nc.sync.dma_start(out=v0_s[:], in_=v0_d[:])
            nc.sync.dma_start(out=wt_s[:], in_=wt_d)
            nc.scalar.copy(it[:, 0, :, :], stg[:, 0, :, :])
            for b in range(BPC):
                nc.vector.tensor_copy(it[0:1, 0, b, :], v0_s[:])
            for k in range(1, 4):
                nc.sync.dma_start(out=stg[:, k, :, :],
                                  in_=src_b2(C * k - 1, C + 1))
                nc.scalar.copy(it[:, k, :, :], stg[:, k, :, :])
            nc.sync.dma_start(out=wt2_s[:], in_=wt2_d)
            nc.sync.dma_start(out=wt0_s[:], in_=wt0_d)

        def chunk_step(rhs_view, carry_dst, evict_to=None, weights=None,
                       carry_eng=None):
            # one chunk: 8 batch-fused matmuls -> fused carry copy ->
            # (optional) fused eviction. rhs_view: [128, BPC, HD] bf16.
            w = wt_s if weights is None else weights
            ps = psum_pool.tile([C + 1, H, BPC, D], f32, tag="ps")
            for hh in range(H):
                nc.tensor.matmul(
                    out=ps[:, hh, :, :],
                    lhsT=w[:, hh, :],
                    rhs=rhs_view[:, :, hh * D:(hh + 1) * D],
                    start=True, stop=True,
                )
            if carry_dst is not None:
                ce = nc.vector.tensor_copy if carry_eng is None else carry_eng
                ce(bhd(carry_dst), ps[0:1, :, :, :])
            if evict_to is not None:
                nc.scalar.copy(bhd(evict_to), ps[0:C, :, :, :])

        def warmup(seg):
            # two zero-carry warm-up chunks re-deriving the carry into
            # SEG_STARTS[seg]; outputs are discarded.
            c0 = SEG_STARTS[seg]
            wA = c0 - 2
            wtile = warm_pool.tile([C + 1, 2, BPC, HD], cdtype, tag="warm",
                                   name=f"warm_{seg}")
            stg = stage_pool.tile([C + 1, 2, BPC, HD], f32, tag="stg")
            for kk in range(2):
                src = bass.AP(
                    tensor=x_d.tensor,
                    offset=x_d.offset + ((wA + kk) * C - 1) * HD,
                    ap=[[HD, C + 1], [T * HD, BPC], [1, HD]],
                )
                nc.sync.dma_start(out=stg[:, kk, :, :], in_=src)
            nc.scalar.copy(wtile[:, :, :, :], stg[:, :, :, :])
            # chunk A uses weights with a zeroed carry K-row, so its junk
            # row 0 contributes nothing (no memset, no chain-engine use)
            chunk_step(wtile[:, 0, :, :], wtile[0:1, 1, :, :],
                       weights=wt0_s)
            g_s, k_s = divmod(c0, 4)
            chunk_step(wtile[:, 1, :, :], in_tiles[g_s][0:1, k_s, :, :])

        # prologue: first two groups + the segment-start groups (warm-up
        # chains must begin early)
        load_group0()
        load_group(1)
        seg_of_group = {}
        for s, c0 in enumerate(SEG_STARTS):
            seg_of_group[c0 // 4] = s

        loaded = {0, 1}

        def ensure_loaded(g):
            if g in loaded:
                return
            loaded.add(g)
            load_group(g)
            if g in seg_of_group:
                warmup(seg_of_group[g])

        for g in (2, 4, 6):
            ensure_loaded(g)

        seg_ends = set(c - 1 for c in SEG_STARTS)
        for g in range(GROUPS):
            ensure_loaded(min(g + 2, GROUPS - 1))
            ensure_loaded(min(g + 3, GROUPS - 1))
            it = in_tiles[g]
            ot = out_pool.tile([C, 4, BPC, HD], f32, tag="out",
                               name=f"ot_{g}")
            for k in range(4):
                chunk = 4 * g + k
                if chunk in seg_ends:
                    carry_dst = None        # next segment re-derives it
                elif k < 3:
                    carry_dst = in_tiles[g][0:1, k + 1, :, :]
                elif g < GROUPS - 1:
                    carry_dst = in_tiles[g + 1][0:1, 0, :, :]
                else:
                    carry_dst = in2_tile[0:1, :, :]
                chunk_step(it[:, k, :, :], carry_dst, ot[:, k, :, :])
            for b in range(BPC):
                dstv = out_d[b, 4 * C * g: 4 * C * (g + 1), :] \
                    .rearrange("(k p) c -> p k c", p=C)
                # un-permute: ot partition 0 = chunk's last time row
                nc.sync.dma_start(out=dstv[0:C - 1, :, :],
                                  in_=ot[1:C, :, b, :])
                nc.sync.dma_start(out=dstv[C - 1:C, :, :],
                                  in_=ot[0:1, :, b, :])

            if g == 4:
                # tail x rows; full-tile cast (row 0 = junk predecessor row)
                # emitted BEFORE g=7's carry copy targets in2_tile row 0
                stg2 = stage_pool.tile([REM + 1, BPC, HD], f32, tag="stg")
                src = bass.AP(
                    tensor=x_d.tensor,
                    offset=x_d.offset + (NFULL * C - 1) * HD,
                    ap=[[HD, REM + 1], [T * HD, BPC], [1, HD]],
                )
                nc.sync.dma_start(out=stg2[:, :, :], in_=src)
                nc.scalar.copy(in2_tile[:, :, :], stg2[:, :, :])

        # --- tail chunk (32 rows) ---
        ps2 = psum2_pool.tile([REM, H, BPC, D], f32, tag="ps2")
        for hh in range(H):
            nc.tensor.matmul(
                out=ps2[:, hh, :, :],
                lhsT=wt2_s[:, hh, :],
                rhs=in2_tile[:, :, hh * D:(hh + 1) * D],
                start=True, stop=True,
            )
        ot2 = out2_pool.tile([REM, BPC, HD], f32, tag="out2", name="ot2")
        nc.scalar.copy(bhd(ot2[:, :, :]), ps2[0:REM, :, :, :])
        for b in range(BPC):
            nc.sync.dma_start(out=out_d[b, NFULL * C:, :], in_=ot2[:, b, :])

    nc.compile()
    return nc


def _get_program():
    key = COMPUTE_DTYPE
    if key not in _cache:
        _cache[key] = _build_program(key)
    return _cache[key]


def kernel(values, smoothing_weight, v0):
    import ml_dtypes
    from concourse.bass_utils import run_bass_kernel_spmd

    np_cdtype = ml_dtypes.bfloat16 if COMPUTE_DTYPE == "bf16" else np.float32
    wt, wt0, wt2, v0row = _host_constants(smoothing_weight, v0, np_cdtype)

    nc = _get_program()
    x = np.ascontiguousarray(values.astype(np.float32).reshape(B, T, HD))
    in_maps = []
    for core in range(NCORES):
        shard = np.ascontiguousarray(x[core * BPC:(core + 1) * BPC])
        in_maps.append({"x": shard, "wt": wt, "wt0": wt0, "wt2": wt2,
                        "v0r": v0row})

    res = run_bass_kernel_spmd(nc, in_maps, list(range(NCORES)))
    outs = [res.results[i]["out"].reshape(BPC, T, H, D)
            for i in range(NCORES)]
    return np.concatenate(outs, axis=0).astype(np.float32)


# revision 29
# speedup vs baseline: 1.4622x; 1.0166x over previous
"""Trainium2 Bass kernel for exponential smoothing (EMA over time).

Math: out[b,t,h,d] = w_h^{t+1} v0[h,d] + sum_{j<=t} (1-w_h) w_h^{t-j} x[b,j,h,d]
(w = sigmoid(smoothing_weight)), i.e. the scan s_t = w s_{t-1} + (1-w) x_t with
s_{-1} = v0.

Kernel strategy (per core, data-parallel over batch: 16 batches / 8 cores,
2 per core):
  - Time is processed in chunks of C=127. Each chunk step runs 8 per-head
    matmuls whose rhs spans BOTH of the core's batches ([128 x (2,64)],
    N=128): rhs row 0 = carry row, rows 1..127 = x rows; lhsT packs the
    decay column w^{p+1} (for the carry) on top of the lower-triangular
    smoothing weights (1-w) w^{p-j}. The *corrected* last output row of a
    chunk IS the carry for the next chunk: cross-chunk propagation is one
    fused [1,1024] PSUM->SBUF row copy per chunk.
  - lhsT columns are permuted so the chunk's last output row sits at PSUM
    partition 0 (engine APs must start 32-aligned); the out-DMA un-permutes.
  - The 33-step carry chain is broken into 4 independent segments: segments
    1..3 re-derive their incoming carry from 2 warm-up chunks computed with
    a zero carry (EMA influence decays as w^dt; w<=~0.95 -> w^254 ~ 1e-5,
    far below bf16 noise). This gives 4 concurrent chains so the PE never
    waits long on a single carry round-trip.
  - 4096 = 32*127 + 32: 32 full chunks + one 32-row tail chunk.
  - Inputs load contiguously via HWDGE (full 128-partition APs with one
    overlapping predecessor row -- misaligned SBUF DMAs serialize onto one
    SDMA engine), are cast f32->bf16 on ACT, matmuls run in bf16 (fp32 PSUM
    accumulate), output evicts to f32 (one fused ACT op per chunk) and
    stores contiguously.
"""

import numpy as np

B, T, H, D = 16, 4096, 8, 64
HD = H * D                    # 512
C = 127                       # chunk length (1 row reserved for the carry)
NFULL = T // C                # 32 full chunks
REM = T - NFULL * C           # 32-row tail chunk
GROUPS = NFULL // 4           # 8 groups of 4 chunks
NCORES = 8
BPC = B // NCORES             # batches per core
SEG_STARTS = [9, 17, 25]      # segment-start chunks (segments 1..3)

COMPUTE_DTYPE = "bf16"        # "bf16" | "fp32"

_cache = {}


def _host_constants(smoothing_weight, v0, np_cdtype):
    """Parameter-derived constants, computed in fp64 on host."""
    w = 1.0 / (1.0 + np.exp(-smoothing_weight.astype(np.float64)))  # [H,1]
    w = w[:, 0]

    def make_lhsT(n):
        # [H, n+1, n]; row 0 = w^(p+1) (carry decay), row 1+j = (1-w) w^(p-j)
        lt = np.zeros((H, n + 1, n), dtype=np.float64)
        p = np.arange(n)
        for hh in range(H):
            lt[hh, 0, :] = w[hh] ** (p + 1)
            for j in range(n):
                lt[hh, 1 + j, j:] = (1.0 - w[hh]) * w[hh] ** (p[j:] - j)
        return lt.astype(np_cdtype)

    wt = make_lhsT(C)          # [H, 128, 127]
    # permute out rows: [last, 0..last-1] so the carry row lands at PSUM
    # partition 0 (aligned); the out-DMA un-permutes
    wt = np.concatenate([wt[:, :, C - 1:], wt[:, :, :C - 1]], axis=2)
    wt2 = make_lhsT(REM)       # [H, 33, 32] (tail: no carry out, unpermuted)
    # pad M to 128 (zero column): Fast Weight Load needs NumWeights == 128;
    # the extra PSUM row is never read
    wt = np.concatenate([wt, np.zeros((H, C + 1, 1), wt.dtype)], axis=2)
    # [K, H, M] layout so the on-chip weight DMA is contiguous per partition
    wt = np.ascontiguousarray(wt.transpose(1, 0, 2))    # [128, 8, 128]
    wt2 = np.ascontiguousarray(wt2.transpose(1, 0, 2))  # [33, 8, 32]
    wt0 = wt.copy()
    wt0[0, :, :] = 0            # K-row 0 (carry) zeroed: warm-up chunk A
    v0row = v0.reshape(1, HD).astype(np_cdtype)   # [1, 512]
    return wt, wt0, wt2, v0row


def _build_program(cdtype_name):
    import concourse.bass as bass
    import concourse.tile as tile
    from concourse import bacc, mybir
    from contextlib import ExitStack

    cdtype = mybir.dt.bfloat16 if cdtype_name == "bf16" else mybir.dt.float32
    f32 = mybir.dt.float32

    nc = bacc.Bacc("TRN2", target_bir_lowering=False, debug=False,
                   num_devices=NCORES)

    x_d = nc.dram_tensor("x", [BPC, T, HD], f32, kind="ExternalInput").ap()
    wt_d = nc.dram_tensor("wt", [C + 1, H, C + 1], cdtype,
                          kind="ExternalInput").ap()
    wt0_d = nc.dram_tensor("wt0", [C + 1, H, C + 1], cdtype,
                           kind="ExternalInput").ap()
    wt2_d = nc.dram_tensor("wt2", [REM + 1, H, REM], cdtype,
                           kind="ExternalInput").ap()
    v0_d = nc.dram_tensor("v0r", [1, HD], cdtype, kind="ExternalInput").ap()
    out_d = nc.dram_tensor("out", [BPC, T, HD], f32, kind="ExternalOutput").ap()

    with tile.TileContext(nc) as tc, ExitStack() as ctx:
        consts = ctx.enter_context(tc.tile_pool(name="consts", bufs=1))
        in_pool = ctx.enter_context(tc.tile_pool(name="inp", bufs=5))
        in2_pool = ctx.enter_context(tc.tile_pool(name="inp2", bufs=1))
        warm_pool = ctx.enter_context(tc.tile_pool(name="warm", bufs=3))
        out_pool = ctx.enter_context(tc.tile_pool(name="outp", bufs=3))
        out2_pool = ctx.enter_context(tc.tile_pool(name="outp2", bufs=2))
        stage_pool = ctx.enter_context(tc.tile_pool(name="stg", bufs=4))
        psum_pool = ctx.enter_context(tc.tile_pool(name="psum", bufs=3,
                                                   space="PSUM"))
        psum2_pool = ctx.enter_context(tc.tile_pool(name="psum2", bufs=1,
                                                    space="PSUM"))

        # --- constants (DMAs issued inside load_group0 for trigger order) ---
        wt_s = consts.tile([C + 1, H, C + 1], cdtype)   # [128, 8, 128]
        wt0_s = consts.tile([C + 1, H, C + 1], cdtype)  # warm-up A weights
        wt2_s = consts.tile([REM + 1, H, REM], cdtype)  # [33, 8, 32]
        v0_s = consts.tile([1, HD], cdtype)

        # --- tile handles (batch-fused: free axis = (chunk, b, hd)) ---
        in_tiles = {g: in_pool.tile([C + 1, 4, BPC, HD], cdtype, tag="in",
                                    name=f"in_{g}")
                    for g in range(GROUPS)}
        in2_tile = in2_pool.tile([REM + 1, BPC, HD], cdtype, tag="in2",
                                 name="in2")

        def bhd(ap):
            # view a [p, b, (h d)] slice as [p, h, b, d] (PSUM layout order)
            return ap.rearrange("p b (h d) -> p h b d", h=H)

        def load_group(g):
            # Emitted BEFORE any carry copy that targets this tile's row 0
            # (the cast covers the full tile; Tile orders same-region writes
            # by program order). Full-128-partition DMAs with an overlapping
            # predecessor row keep the SBUF side port-group aligned.
            it = in_tiles[g]
            stg = stage_pool.tile([C + 1, 4, BPC, HD], f32, tag="stg")
            for b in range(BPC):
                xb = x_d[b]
                src = bass.AP(
                    tensor=xb.tensor,
                    offset=xb.offset + (4 * C * g - 1) * HD,
                    ap=[[HD, C + 1], [C * HD, 4], [1, HD]],
                )
                nc.sync.dma_start(out=stg[:, :, b, :], in_=src)
            nc.scalar.copy(it[:, :, :, :], stg[:, :, :, :])

        def load_group0():
            # Fast start: per-chunk loads + casts so chunk 0's matmuls can
            # begin as soon as ~256KB has landed (k0 DMAs + v0 + weights are
            # the very first triggers on the serial SP queue). Chunk 0 has
            # no predecessor row: rows split into an unaligned 31-row piece,
            # an aligned 96-row piece, and a junk row 0.
            it = in_tiles[0]
            stg = stage_pool.tile([C + 1, 4, BPC, HD], f32, tag="stg")

            def src_b2(row0, nrows):
                return bass.AP(
                    tensor=x_d.tensor,
                    offset=x_d.offset + row0 * HD,
                    ap=[[HD, nrows], [T * HD, BPC], [1, HD]],
                )
            nc.sync.dma_start(out=stg[0:1, 0, :, :], in_=src_b2(0, 1))
            nc.sync.dma_start(out=stg[1:32, 0, :, :], in_=src_b2(0, 31))
            nc.sync.dma_start(out=stg[32:C + 1, 0, :, :], in_=src_b2(31, 96))
            nc.sync.dma_start(out=v0_s[:], in_=v0_d[:])
            nc.sync.dma_start(out=wt_s[:], in_=wt_d)
            nc.scalar.copy(it[:, 0, :, :], stg[:, 0, :, :])
            for b in range(BPC):
                nc.vector.tensor_copy(it[0:1, 0, b, :], v0_s[:])
            for k in range(1, 4):
                nc.sync.dma_start(out=stg[:, k, :, :],
                                  in_=src_b2(C * k - 1, C + 1))
                nc.scalar.copy(it[:, k, :, :], stg[:, k, :, :])
            nc.sync.dma_start(out=wt2_s[:], in_=wt2_d)
            nc.sync.dma_start(out=wt0_s[:], in_=wt0_d)

        def chunk_step(rhs_view, carry_dst, evict_to=None, weights=None,
                       carry_eng=None):
            # one chunk: 8 batch-fused matmuls -> fused carry copy ->
            # (optional) fused eviction. rhs_view: [128, BPC, HD] bf16.
            w = wt_s if weights is None else weights
            ps = psum_pool.tile([C + 1, H, BPC, D], f32, tag="ps")
            for hh in range(H):
                nc.tensor.matmul(
                    out=ps[:, hh, :, :],
                    lhsT=w[:, hh, :],
                    rhs=rhs_view[:, :, hh * D:(hh + 1) * D],
                    start=True, stop=True,
                )
            if carry_dst is not None:
                ce = nc.vector.tensor_copy if carry_eng is None else carry_eng
                ce(bhd(carry_dst), ps[0:1, :, :, :])
            if evict_to is not None:
                nc.scalar.copy(bhd(evict_to), ps[0:C, :, :, :])

        def warmup(seg):
            # two zero-carry warm-up chunks re-deriving the carry into
            # SEG_STARTS[seg]; outputs are discarded.
            c0 = SEG_STARTS[seg]
            wA = c0 - 2
            wtile = warm_pool.tile([C + 1, 2, BPC, HD], cdtype, tag="warm",
                                   name=f"warm_{seg}")
            stg = stage_pool.tile([C + 1, 2, BPC, HD], f32, tag="stg")
            for kk in range(2):
                src = bass.AP(
                    tensor=x_d.tensor,
                    offset=x_d.offset + ((wA + kk) * C - 1) * HD,
                    ap=[[HD, C + 1], [T * HD, BPC], [1, HD]],
                )
                nc.sync.dma_start(out=stg[:, kk, :, :], in_=src)
            nc.scalar.copy(wtile[:, :, :, :], stg[:, :, :, :])
            # chunk A uses weights with a zeroed carry K-row, so its junk
            # row 0 contributes nothing (no memset, no chain-engine use)
            chunk_step(wtile[:, 0, :, :], wtile[0:1, 1, :, :],
                       weights=wt0_s)
            g_s, k_s = divmod(c0, 4)
            chunk_step(wtile[:, 1, :, :], in_tiles[g_s][0:1, k_s, :, :])

        # prologue: first two groups + the segment-start groups (warm-up
        # chains must begin early)
        load_group0()
        load_group(1)
        seg_of_group = {}
        for s, c0 in enumerate(SEG_STARTS):
            seg_of_group[c0 // 4] = s

        loaded = {0, 1}

        def ensure_loaded(g):
            if g in loaded:
                return
            loaded.add(g)
            load_group(g)
            if g in seg_of_group:
                warmup(seg_of_group[g])

        for g in (2, 4, 6):
            ensure_loaded(g)

        seg_ends = set(c - 1 for c in SEG_STARTS)
        for g in range(GROUPS):
            ensure_loaded(min(g + 2, GROUPS - 1))
            ensure_loaded(min(g + 3, GROUPS - 1))
            it = in_tiles[g]
            ot = out_pool.tile([C, 4, BPC, HD], f32, tag="out",
                               name=f"ot_{g}")
            for k in range(4):
                chunk = 4 * g + k
                if chunk in seg_ends:
                    carry_dst = None        # next segment re-derives it
                elif k < 3:
                    carry_dst = in_tiles[g][0:1, k + 1, :, :]
                elif g < GROUPS - 1:
                    carry_dst = in_tiles[g + 1][0:1, 0, :, :]
                else:
                    carry_dst = in2_tile[0:1, :, :]
                chunk_step(it[:, k, :, :], carry_dst, ot[:, k, :, :])
            for b in range(BPC):
                dstv = out_d[b, 4 * C * g: 4 * C * (g + 1), :] \
                    .rearrange("(k p) c -> p k c", p=C)
                # un-permute: ot partition 0 = chunk's last time row
                nc.sync.dma_start(out=dstv[0:C - 1, :, :],
                                  in_=ot[1:C, :, b, :])
                nc.sync.dma_start(out=dstv[C - 1:C, :, :],
                                  in_=ot[0:1, :, b, :])

            if g == 4:
                # tail x rows; full-tile cast (row 0 = junk predecessor row)
                # emitted BEFORE g=7's carry copy targets in2_tile row 0
                stg2 = stage_pool.tile([REM + 1, BPC, HD], f32, tag="stg")
                src = bass.AP(
                    tensor=x_d.tensor,
                    offset=x_d.offset + (NFULL * C - 1) * HD,
                    ap=[[HD, REM + 1], [T * HD, BPC], [1, HD]],
                )
                nc.sync.dma_start(out=stg2[:, :, :], in_=src)
                nc.scalar.copy(in2_tile[:, :, :], stg2[:, :, :])

        # --- tail chunk (32 rows) ---
        ps2 = psum2_pool.tile([REM, H, BPC, D], f32, tag="ps2")
        for hh in range(H):
            nc.tensor.matmul(
                out=ps2[:, hh, :, :],
                lhsT=wt2_s[:, hh, :],
                rhs=in2_tile[:, :, hh * D:(hh + 1) * D],
                start=True, stop=True,
            )
        ot2 = out2_pool.tile([REM, BPC, HD], f32, tag="out2", name="ot2")
        nc.scalar.copy(bhd(ot2[:, :, :]), ps2[0:REM, :, :, :])
        for b in range(BPC):
            nc.sync.dma_start(out=out_d[b, NFULL * C:, :], in_=ot2[:, b, :])

    nc.compile()
    return nc


def _get_program():
    key = COMPUTE_DTYPE
    if key not in _cache:
        _cache[key] = _build_program(key)
    return _cache[key]


def kernel(values, smoothing_weight, v0):
    import ml_dtypes
    from concourse.bass_utils import run_bass_kernel_spmd

    np_cdtype = ml_dtypes.bfloat16 if COMPUTE_DTYPE == "bf16" else np.float32
    wt, wt0, wt2, v0row = _host_constants(smoothing_weight, v0, np_cdtype)

    nc = _get_program()
    x = np.ascontiguousarray(values.astype(np.float32).reshape(B, T, HD))
    in_maps = []
    for core in range(NCORES):
        shard = np.ascontiguousarray(x[core * BPC:(core + 1) * BPC])
        in_maps.append({"x": shard, "wt": wt, "wt0": wt0, "wt2": wt2,
                        "v0r": v0row})

    res = run_bass_kernel_spmd(nc, in_maps, list(range(NCORES)))
    outs = [res.results[i]["out"].reshape(BPC, T, H, D)
            for i in range(NCORES)]
    return np.concatenate(outs, axis=0).astype(np.float32)
